# revision 9
# baseline (speedup 1.0000x reference)
"""MiMoV2 MoE gate (moe_routing) on 8 Trainium2 NeuronCores.

v2 strategy (vs v1's 3x fp16 x-stationary scheme):
  - Token-shard across 8 cores (2048 tokens each); replicate gate weight.
  - Gating GEMM with W STATIONARY and x MOVING (out = [expert, token]):
    LDWEIGHTS amortizes over N=512 moving tokens instead of 256 experts,
    so the PE streams matmuls at full rate.
  - 2 PE-units of work instead of 3: hi term (fp16(x)*2^8)@(fp16(W)*2^8)
    plus ONE DoubleRow fp8 matmul per chunk computing BOTH cross terms:
      plane0: e4m3(x1) @ e4m3((W - W1)*2^16)      [x-hi  * W-residual]
      plane1: e4m3((x-x1)*2^11) @ e4m3(W1*2^5)    [x-res * W-hi      ]
    All products land at 2^16*logits and accumulate into ONE PSUM bank.
    CPU-sim of this exact arithmetic: logit rmse 1.3e-5 -> 7/16384 tokens
    flip (rel err 8.3e-3, gate is 2e-2).
  - sigmoid directly from PSUM on ScalarE: Sigmoid(psum * 2^-16).
  - PE-transpose (identity matmul) of sigmoid scores back to [token, expert]
    layout, then the v1 routing pipeline (group top-2 via match_replace,
    top-4 groups, masked max8/max_index) with ops split across DVE /
    ScalarE / GpSimd so no single engine binds.

Inputs (full):  hidden_states [4,4096,4096] f32, weight [256,4096] f32,
                e_score_correction_bias [256] f32
Output (full):  (topk_idx [16384,8] int32, topk_weight [16384,8] f32)
"""

import numpy as np
import ml_dtypes

import concourse.tile as tile
from concourse import bacc, mybir
from concourse.bass_utils import run_bass_kernel_spmd

# problem shape (hardcoded per contract)
T_FULL = 16384
H = 4096
E = 256
G = 8
GS = E // G           # 32
TOPK = 8
SCALING = 2.5

N_CORES = 8
T_CORE = T_FULL // N_CORES    # 2048
NCHK = H // 128               # 32 contraction chunks
EH = 2                        # expert halves (stationary is 128 experts)
TB = 512                      # token block (moving free dim)
NB = T_CORE // TB             # 4 blocks
NQ = 4                        # x DMA pieces per block (chunk quarters)
QC = NCHK // NQ               # 8 chunks per piece

F8 = ml_dtypes.float8_e4m3    # TRN e4m3 (IEEE: max 240, has inf) == dt.float8e4

_BUILT = None


def _build():
    f32 = mybir.dt.float32
    f16 = mybir.dt.float16
    f8 = mybir.dt.float8e4
    u32 = mybir.dt.uint32
    AF = mybir.ActivationFunctionType
    OP = mybir.AluOpType
    AX = mybir.AxisListType
    DR = mybir.MatmulPerfMode.DoubleRow

    nc = bacc.Bacc("TRN2", target_bir_lowering=False, debug=False)

    # x1s: fp16(x)*2^8, per block laid [128p, c, t] contiguous
    x1s = nc.dram_tensor("x1s", [NB, 128, NCHK * TB], f16, kind="ExternalInput").ap()
    # xc8: fp8 pair planes per chunk: [128p, c, {e4m3(x1), e4m3((x-x1)*2^11)}, t]
    xc8 = nc.dram_tensor("xc8", [NB, 128, NCHK * 2 * TB], f8, kind="ExternalInput").ap()
    # w1s: fp16(W)*2^8 as [128k, c, ehalf, e]
    w1s = nc.dram_tensor("w1s", [128, NCHK * EH * 128], f16, kind="ExternalInput").ap()
    # wc8: fp8 pair planes: [128k, c, ehalf, {e4m3(w2*2^16), e4m3(w1*2^5)}, e]
    wc8 = nc.dram_tensor("wc8", [128, NCHK * EH * 2 * 128], f8, kind="ExternalInput").ap()
    bias_rep = nc.dram_tensor("bias_rep", [128, E], f32, kind="ExternalInput").ap()
    ident = nc.dram_tensor("ident", [128, 128], f32, kind="ExternalInput").ap()

    idx_out = nc.dram_tensor("idx_out", [T_CORE, TOPK], u32, kind="ExternalOutput").ap()
    w_out = nc.dram_tensor("w_out", [T_CORE, TOPK], f32, kind="ExternalOutput").ap()

    w1v = w1s.rearrange("p (c h e) -> p c h e", c=NCHK, h=EH)
    wc8v = wc8.rearrange("p (c h l e) -> p c h l e", c=NCHK, h=EH, l=2)

    with tile.TileContext(nc) as tc:
        with tc.tile_pool(name="const", bufs=1) as cpool, \
             tc.tile_pool(name="xin", bufs=2) as xpool, \
             tc.tile_pool(name="srt", bufs=2) as rpool, \
             tc.tile_pool(name="mid", bufs=5) as mpool, \
             tc.tile_pool(name="small", bufs=8) as qpool, \
             tc.tile_pool(name="psum", bufs=3, space="PSUM") as ppool, \
             tc.tile_pool(name="psumt", bufs=5, space="PSUM") as tpool:

            # constants: W in chunk-quarters (first quarter unblocks first MMs)
            W1t = cpool.tile([128, NCHK, EH, 128], f16, tag="W1t")
            Wc8t = cpool.tile([128, NCHK, EH, 2, 128], f8, tag="Wc8t")
            IDT = cpool.tile([128, 128], f32, tag="IDT")
            BR = cpool.tile([128, E], f32, tag="BR")
            for q in range(NQ):
                cs = slice(q * QC, (q + 1) * QC)
                nc.sync.dma_start(W1t[:, cs], w1v[:, cs])
                nc.sync.dma_start(Wc8t[:, cs], wc8v[:, cs])
                if q == 0:
                    nc.sync.dma_start(IDT[:], ident)
                    nc.sync.dma_start(BR[:], bias_rep)

            srTs = {}

            def emit_mm_block(bi):
                # x DMA in chunk-quarter pieces so first MMs start early
                x1q = []
                x8q = []
                for q in range(NQ):
                    t1 = xpool.tile([128, QC, TB], f16, tag=f"x1q{q}")
                    t8 = xpool.tile([128, QC, 2, TB], f8, tag=f"x8q{q}")
                    seg1 = x1s[bi][:, q * QC * TB:(q + 1) * QC * TB]
                    seg8 = xc8[bi][:, q * QC * 2 * TB:(q + 1) * QC * 2 * TB]
                    nc.sync.dma_start(t1[:], seg1.rearrange("p (c t) -> p c t", c=QC))
                    nc.sync.dma_start(t8[:], seg8.rearrange("p (c l t) -> p c l t",
                                                            c=QC, l=2))
                    x1q.append(t1)
                    x8q.append(t8)

                # logits (scaled 2^16) into one PSUM bank per expert-half
                srT = rpool.tile([128, EH, TB], f32, tag="srT")
                srTs[bi] = srT
                for eh in range(EH):
                    ps = ppool.tile([128, TB], f32, tag="ps")
                    for c in range(NCHK):
                        q, ci = divmod(c, QC)
                        nc.tensor.matmul(ps[:], W1t[:, c, eh, :], x1q[q][:, ci, :],
                                         start=(c == 0), stop=False)
                        nc.tensor.matmul(ps[:], Wc8t[:, c, eh], x8q[q][:, ci],
                                         perf_mode=DR,
                                         start=False, stop=(c == NCHK - 1))
                    # sigmoid straight off PSUM (Exp-LUT-accurate enough at our
                    # 1.3e-5 logit noise); output is raw scores, expert-major
                    nc.scalar.activation(srT[:, eh], ps[:], AF.Sigmoid,
                                         scale=float(2.0 ** -16))

            def emit_routing_block(bi):
                srT = srTs.pop(bi)
                NT = TB // 128
                tgs = [dict() for _ in range(NT)]

                # step 0: transposes (eh0 batch first: it is ready before eh1)
                for tg in range(NT):
                    tgs[tg]["psT"] = tpool.tile([128, E], f32, name="psT", tag="psT")
                for eh in range(EH):
                    for tg in range(NT):
                        nc.tensor.transpose(
                            tgs[tg]["psT"][:, eh * 128:(eh + 1) * 128],
                            srT[:, eh, tg * 128:(tg + 1) * 128], IDT[:])

                def step(fn):
                    for tg in range(NT):
                        fn(tgs[tg], bi * TB + tg * 128)

                # interleave the 4 token-groups per pipeline step so each
                # engine always has 3 other groups' work while one group
                # round-trips to another engine
                def s_cho(t, tok0):
                    t["s_cho"] = mpool.tile([128, E], f32, name="s_cho", tag="s_cho")
                    nc.vector.tensor_add(t["s_cho"][:], t["psT"][:], BR[:])

                def m1(t, tok0):
                    t["m1"] = qpool.tile([128, G], f32, name="m1", tag="m1")
                    nc.vector.reduce_max(
                        t["m1"][:], t["s_cho"][:].rearrange("p (g s) -> p g s", g=G),
                        axis=AX.X)

                def repl(t, tok0):
                    t["repl"] = mpool.tile([128, E], f32, name="repl", tag="repl")
                    nc.vector.match_replace(t["repl"][:], t["m1"][:], t["s_cho"][:],
                                            -1e30)

                def m2(t, tok0):
                    t["m2"] = qpool.tile([128, G], f32, name="m2", tag="m2")
                    nc.vector.reduce_max(
                        t["m2"][:], t["repl"][:].rearrange("p (g s) -> p g s", g=G),
                        axis=AX.X)

                def gsum(t, tok0):
                    t["gsum"] = qpool.tile([128, G], f32, name="gsum", tag="gsum")
                    nc.gpsimd.tensor_add(t["gsum"][:], t["m1"][:], t["m2"][:])

                def gs8(t, tok0):
                    t["gs8"] = qpool.tile([128, 8], f32, name="gs8", tag="gs8")
                    nc.vector.max(t["gs8"][:], t["gsum"][:])

                def pen(t, tok0):
                    # pen = (gsum < 4th) * -1e30: -0.0 for kept groups, so kept
                    # scores pass through bit-exact
                    t["pen"] = qpool.tile([128, G], f32, name="pen", tag="pen")
                    nc.vector.tensor_scalar(t["pen"][:], t["gsum"][:],
                                            t["gs8"][:, 3:4], -1e30,
                                            op0=OP.is_lt, op1=OP.mult)

                def s_mask(t, tok0):
                    t["s_mask"] = mpool.tile([128, E], f32, name="s_mask", tag="s_mask")
                    pen_b = t["pen"][:].unsqueeze(2).broadcast_to([128, G, GS])
                    nc.gpsimd.tensor_tensor(
                        t["s_mask"][:].rearrange("p (g s) -> p g s", g=G),
                        t["s_cho"][:].rearrange("p (g s) -> p g s", g=G),
                        pen_b, op=OP.add)

                def v8(t, tok0):
                    t["v8"] = qpool.tile([128, 8], f32, name="v8", tag="v8")
                    nc.vector.max(t["v8"][:], t["s_mask"][:])

                def i8(t, tok0):
                    t["i8"] = qpool.tile([128, 8], u32, name="i8", tag="i8")
                    nc.vector.max_index(t["i8"][:], t["v8"][:], t["s_mask"][:])
                    nc.scalar.dma_start(idx_out[tok0:tok0 + 128, :], t["i8"][:])

                def sel2(t, tok0):
                    t["sel2"] = mpool.tile([128, E], f32, name="sel2", tag="sel2")
                    nc.vector.tensor_scalar(t["sel2"][:], t["s_mask"][:],
                                            t["v8"][:, 7:8], -1.0,
                                            op0=OP.is_ge, op1=OP.add)

                def r_sel(t, tok0):
                    # raw scores (psT) masked to the selected 8
                    t["r_sel"] = mpool.tile([128, E], f32, name="r_sel", tag="r_sel")
                    nc.vector.scalar_tensor_tensor(t["r_sel"][:], in0=t["sel2"][:],
                                                   scalar=1e30, in1=t["psT"][:],
                                                   op0=OP.mult, op1=OP.add)

                def w8d(t, tok0):
                    t["w8d"] = qpool.tile([128, 8], f32, name="w8d", tag="w8d")
                    nc.vector.max(t["w8d"][:], t["r_sel"][:])

                def ri8(t, tok0):
                    t["ri8"] = qpool.tile([128, 8], u32, name="ri8", tag="ri8")
                    nc.vector.max_index(t["ri8"][:], t["w8d"][:], t["r_sel"][:])

                def eq64(t, tok0):
                    t["eq64"] = qpool.tile([128, 8, 8], f32, name="eq64", tag="eq64")
                    i8_b = t["i8"][:].unsqueeze(2).broadcast_to([128, 8, 8])
                    ri8_b = t["ri8"][:].unsqueeze(1).broadcast_to([128, 8, 8])
                    nc.vector.tensor_tensor(t["eq64"][:], i8_b, ri8_b,
                                            op=OP.is_equal)

                def w64(t, tok0):
                    t["w64"] = qpool.tile([128, 8, 8], f32, name="w64", tag="w64")
                    w8d_b = t["w8d"][:].unsqueeze(1).broadcast_to([128, 8, 8])
                    nc.gpsimd.tensor_tensor(t["w64"][:], t["eq64"][:], w8d_b,
                                            op=OP.mult)

                def w8p(t, tok0):
                    t["w8p"] = qpool.tile([128, 8], f32, name="w8p", tag="w8p")
                    nc.vector.reduce_sum(t["w8p"][:], t["w64"][:], axis=AX.X)

                def sum8(t, tok0):
                    t["sum8"] = qpool.tile([128, 1], f32, name="sum8", tag="sum8")
                    nc.vector.reduce_sum(t["sum8"][:], t["w8p"][:], axis=AX.X)

                def rcp(t, tok0):
                    t["rcp"] = qpool.tile([128, 1], f32, name="rcp", tag="rcp")
                    nc.vector.reciprocal(t["rcp"][:], t["sum8"][:])

                def wf(t, tok0):
                    t["wf"] = qpool.tile([128, 8], f32, name="wf", tag="wf")
                    nc.vector.tensor_scalar(t["wf"][:], t["w8p"][:],
                                            t["rcp"][:, 0:1], SCALING,
                                            op0=OP.mult, op1=OP.mult)
                    nc.scalar.dma_start(w_out[tok0:tok0 + 128, :], t["wf"][:])

                for fn in (s_cho, m1, repl, m2, gsum, gs8, pen, s_mask, v8, i8,
                           sel2, r_sel, w8d, ri8, eq64, w64, w8p, sum8, rcp, wf):
                    step(fn)

            # software pipeline: routing for block b-1 is emitted BEFORE block
            # b's matmuls, so the b-1 transposes sit ahead of MM(b) in the
            # in-order PE queue and routing overlaps the MM stream
            for bi in range(NB + 1):
                if bi >= 1:
                    emit_routing_block(bi - 1)
                if bi < NB:
                    emit_mm_block(bi)

    nc.compile()
    return nc


def _get_built():
    global _BUILT
    if _BUILT is None:
        _BUILT = _build()
    return _BUILT


def _e4m3(a):
    return np.clip(a, -240.0, 240.0).astype(F8)


def _prep_in_maps(hidden_states, weight, e_score_correction_bias):
    x = np.asarray(hidden_states, dtype=np.float32).reshape(T_FULL, H)
    W = np.asarray(weight, dtype=np.float32)
    b = np.asarray(e_score_correction_bias, dtype=np.float32)

    # ---- weights (shared by all cores) ----
    Wt = np.ascontiguousarray(W.T)                       # [H, E]
    w1 = Wt.astype(np.float16)
    w1f = w1.astype(np.float32)
    w2 = Wt - w1f
    w1s_full = (w1f * 256.0).astype(np.float16)          # exact pow2 scale
    # [H, E] -> [128, c, eh, e]
    w1s_host = np.ascontiguousarray(
        w1s_full.reshape(NCHK, 128, EH, 128).transpose(1, 0, 2, 3)
    ).reshape(128, NCHK * EH * 128)
    wc8_pl0 = _e4m3(w2 * np.float32(2.0 ** 16))          # [H, E]
    wc8_pl1 = _e4m3(w1f * np.float32(2.0 ** 5))
    wc8_st = np.stack([wc8_pl0.reshape(NCHK, 128, EH, 128),
                       wc8_pl1.reshape(NCHK, 128, EH, 128)], axis=3)  # c,p,eh,l,e
    wc8_host = np.ascontiguousarray(wc8_st.transpose(1, 0, 2, 3, 4)).reshape(
        128, NCHK * EH * 2 * 128)

    bias_rep = np.ascontiguousarray(np.tile(b[None, :], (128, 1)))
    ident = np.eye(128, dtype=np.float32)

    # ---- per-core x ----
    in_maps = []
    for core in range(N_CORES):
        sl = slice(core * T_CORE, (core + 1) * T_CORE)
        xT = np.ascontiguousarray(x[sl].T)               # [H, 2048]
        x1 = xT.astype(np.float16)
        x1f = x1.astype(np.float32)
        x2 = xT - x1f
        x1s_full = (x1f * 256.0).astype(np.float16)      # [H, 2048]
        x1_8 = _e4m3(x1f)
        x2_8 = _e4m3(x2 * np.float32(2.0 ** 11))

        # [H, T] -> [NB, 128, c*t] (block-major, chunk-major within block)
        v = x1s_full.reshape(NCHK, 128, NB, TB)
        x1s_host = np.ascontiguousarray(v.transpose(2, 1, 0, 3)).reshape(
            NB, 128, NCHK * TB)
        v8 = np.stack([x1_8.reshape(NCHK, 128, NB, TB),
                       x2_8.reshape(NCHK, 128, NB, TB)], axis=1)  # c,l,p,b,t
        xc8_host = np.ascontiguousarray(v8.transpose(3, 2, 0, 1, 4)).reshape(
            NB, 128, NCHK * 2 * TB)

        in_maps.append({
            "x1s": x1s_host, "xc8": xc8_host,
            "w1s": w1s_host, "wc8": wc8_host,
            "bias_rep": bias_rep, "ident": ident,
        })
    return in_maps


def kernel(hidden_states: np.ndarray, weight: np.ndarray,
           e_score_correction_bias: np.ndarray):
    in_maps = _prep_in_maps(hidden_states, weight, e_score_correction_bias)
    nc = _get_built()
    res = run_bass_kernel_spmd(nc, in_maps, list(range(N_CORES)))

    idx = np.concatenate([r["idx_out"] for r in res.results], axis=0).astype(np.int32)
    w = np.concatenate([r["w_out"] for r in res.results], axis=0).astype(np.float32)
    return idx, w


# revision 11
# speedup vs baseline: 1.1640x; 1.1640x over previous
"""MiMoV2 MoE gate (moe_routing) on 8 Trainium2 NeuronCores.

v2 strategy (vs v1's 3x fp16 x-stationary scheme):
  - Token-shard across 8 cores (2048 tokens each); replicate gate weight.
  - Gating GEMM with W STATIONARY and x MOVING (out = [expert, token]):
    LDWEIGHTS amortizes over N=512 moving tokens instead of 256 experts,
    so the PE streams matmuls at full rate.
  - 2 PE-units of work instead of 3: hi term (fp16(x)*2^8)@(fp16(W)*2^8)
    plus ONE DoubleRow fp8 matmul per chunk computing BOTH cross terms:
      plane0: e4m3(x1) @ e4m3((W - W1)*2^16)      [x-hi  * W-residual]
      plane1: e4m3((x-x1)*2^11) @ e4m3(W1*2^5)    [x-res * W-hi      ]
    All products land at 2^16*logits and accumulate into ONE PSUM bank.
    CPU-sim of this exact arithmetic: logit rmse 1.3e-5 -> 7/16384 tokens
    flip (rel err 8.3e-3, gate is 2e-2).
  - sigmoid directly from PSUM on ScalarE: Sigmoid(psum * 2^-16).
  - PE-transpose (identity matmul) of sigmoid scores back to [token, expert]
    layout, then the v1 routing pipeline (group top-2 via match_replace,
    top-4 groups, masked max8/max_index) with ops split across DVE /
    ScalarE / GpSimd so no single engine binds.

Inputs (full):  hidden_states [4,4096,4096] f32, weight [256,4096] f32,
                e_score_correction_bias [256] f32
Output (full):  (topk_idx [16384,8] int32, topk_weight [16384,8] f32)
"""

import numpy as np
import ml_dtypes

import concourse.tile as tile
from concourse import bacc, mybir
from concourse.bass_utils import run_bass_kernel_spmd

# problem shape (hardcoded per contract)
T_FULL = 16384
H = 4096
E = 256
G = 8
GS = E // G           # 32
TOPK = 8
SCALING = 2.5

N_CORES = 8
T_CORE = T_FULL // N_CORES    # 2048
NCHK = H // 128               # 32 contraction chunks
EH = 2                        # expert halves (stationary is 128 experts)
TB = 512                      # token block (moving free dim)
NB = T_CORE // TB             # 4 blocks
NQ = 4                        # x DMA pieces per block (chunk quarters)
QC = NCHK // NQ               # 8 chunks per piece

F8 = ml_dtypes.float8_e4m3    # TRN e4m3 (IEEE: max 240, has inf) == dt.float8e4

_BUILT = None


def _build():
    f32 = mybir.dt.float32
    f16 = mybir.dt.float16
    f8 = mybir.dt.float8e4
    u32 = mybir.dt.uint32
    AF = mybir.ActivationFunctionType
    OP = mybir.AluOpType
    AX = mybir.AxisListType
    DR = mybir.MatmulPerfMode.DoubleRow

    nc = bacc.Bacc("TRN2", target_bir_lowering=False, debug=False)

    # x1s: fp16(x)*2^8, per block laid [128p, c, t] contiguous
    x1s = nc.dram_tensor("x1s", [NB, 128, NCHK * TB], f16, kind="ExternalInput").ap()
    # xc8: fp8 pair planes per chunk: [128p, c, {e4m3(x1), e4m3((x-x1)*2^11)}, t]
    xc8 = nc.dram_tensor("xc8", [NB, 128, NCHK * 2 * TB], f8, kind="ExternalInput").ap()
    # w1s: fp16(W)*2^8 as [128k, c, ehalf, e]
    w1s = nc.dram_tensor("w1s", [128, NCHK * EH * 128], f16, kind="ExternalInput").ap()
    # wc8: fp8 pair planes: [128k, c, ehalf, {e4m3(w2*2^16), e4m3(w1*2^5)}, e]
    wc8 = nc.dram_tensor("wc8", [128, NCHK * EH * 2 * 128], f8, kind="ExternalInput").ap()
    bias_rep = nc.dram_tensor("bias_rep", [128, E], f32, kind="ExternalInput").ap()
    ident = nc.dram_tensor("ident", [128, 128], f32, kind="ExternalInput").ap()

    idx_out = nc.dram_tensor("idx_out", [T_CORE, TOPK], u32, kind="ExternalOutput").ap()
    w_out = nc.dram_tensor("w_out", [T_CORE, TOPK], f32, kind="ExternalOutput").ap()

    w1v = w1s.rearrange("p (c h e) -> p c h e", c=NCHK, h=EH)
    wc8v = wc8.rearrange("p (c h l e) -> p c h l e", c=NCHK, h=EH, l=2)

    with tile.TileContext(nc) as tc:
        with tc.tile_pool(name="const", bufs=1) as cpool, \
             tc.tile_pool(name="xin", bufs=2) as xpool, \
             tc.tile_pool(name="srt", bufs=2) as rpool, \
             tc.tile_pool(name="mid", bufs=5) as mpool, \
             tc.tile_pool(name="small", bufs=8) as qpool, \
             tc.tile_pool(name="psum", bufs=3, space="PSUM") as ppool, \
             tc.tile_pool(name="psumt", bufs=5, space="PSUM") as tpool:

            # constants: W in chunk-quarters (first quarter unblocks first MMs)
            W1t = cpool.tile([128, NCHK, EH, 128], f16, tag="W1t")
            Wc8t = cpool.tile([128, NCHK, EH, 2, 128], f8, tag="Wc8t")
            IDT = cpool.tile([128, 128], f32, tag="IDT")
            BR = cpool.tile([128, E], f32, tag="BR")
            for q in range(NQ):
                cs = slice(q * QC, (q + 1) * QC)
                nc.sync.dma_start(W1t[:, cs], w1v[:, cs])
                nc.sync.dma_start(Wc8t[:, cs], wc8v[:, cs])
                if q == 0:
                    nc.sync.dma_start(IDT[:], ident)
                    nc.sync.dma_start(BR[:], bias_rep)

            srTs = {}

            def emit_mm_block(bi):
                # x DMA in chunk-quarter pieces so first MMs start early
                x1q = []
                x8q = []
                for q in range(NQ):
                    t1 = xpool.tile([128, QC, TB], f16, tag=f"x1q{q}")
                    t8 = xpool.tile([128, QC, 2, TB], f8, tag=f"x8q{q}")
                    seg1 = x1s[bi][:, q * QC * TB:(q + 1) * QC * TB]
                    seg8 = xc8[bi][:, q * QC * 2 * TB:(q + 1) * QC * 2 * TB]
                    nc.sync.dma_start(t1[:], seg1.rearrange("p (c t) -> p c t", c=QC))
                    nc.sync.dma_start(t8[:], seg8.rearrange("p (c l t) -> p c l t",
                                                            c=QC, l=2))
                    x1q.append(t1)
                    x8q.append(t8)

                # logits (scaled 2^16) into one PSUM bank per expert-half
                srT = rpool.tile([128, EH, TB], f32, tag="srT")
                srTs[bi] = srT
                for eh in range(EH):
                    ps = ppool.tile([128, TB], f32, tag="ps")
                    for c in range(NCHK):
                        q, ci = divmod(c, QC)
                        nc.tensor.matmul(ps[:], W1t[:, c, eh, :], x1q[q][:, ci, :],
                                         start=(c == 0), stop=False)
                        nc.tensor.matmul(ps[:], Wc8t[:, c, eh], x8q[q][:, ci],
                                         perf_mode=DR,
                                         start=False, stop=(c == NCHK - 1))
                    # sigmoid straight off PSUM (Exp-LUT-accurate enough at our
                    # 1.3e-5 logit noise); output is raw scores, expert-major
                    nc.scalar.activation(srT[:, eh], ps[:], AF.Sigmoid,
                                         scale=float(2.0 ** -16))

            def emit_routing_block(bi):
                srT = srTs.pop(bi)
                NT = TB // 128
                tgs = [dict() for _ in range(NT)]

                # step 0: transposes (eh0 batch first: it is ready before eh1)
                for tg in range(NT):
                    tgs[tg]["psT"] = tpool.tile([128, E], f32, name="psT", tag="psT")
                for eh in range(EH):
                    for tg in range(NT):
                        nc.tensor.transpose(
                            tgs[tg]["psT"][:, eh * 128:(eh + 1) * 128],
                            srT[:, eh, tg * 128:(tg + 1) * 128], IDT[:])

                def step(fn):
                    for tg in range(NT):
                        fn(tgs[tg], bi * TB + tg * 128)

                # interleave the 4 token-groups per pipeline step so each
                # engine always has 3 other groups' work while one group
                # round-trips to another engine
                def s_cho(t, tok0):
                    t["s_cho"] = mpool.tile([128, E], f32, name="s_cho", tag="s_cho")
                    nc.vector.tensor_add(t["s_cho"][:], t["psT"][:], BR[:])

                def m1(t, tok0):
                    t["m1"] = qpool.tile([128, G], f32, name="m1", tag="m1")
                    nc.vector.reduce_max(
                        t["m1"][:], t["s_cho"][:].rearrange("p (g s) -> p g s", g=G),
                        axis=AX.X)

                def repl(t, tok0):
                    t["repl"] = mpool.tile([128, E], f32, name="repl", tag="repl")
                    nc.vector.match_replace(t["repl"][:], t["m1"][:], t["s_cho"][:],
                                            -1e30)

                def m2(t, tok0):
                    t["m2"] = qpool.tile([128, G], f32, name="m2", tag="m2")
                    nc.vector.reduce_max(
                        t["m2"][:], t["repl"][:].rearrange("p (g s) -> p g s", g=G),
                        axis=AX.X)

                def gsum(t, tok0):
                    t["gsum"] = qpool.tile([128, G], f32, name="gsum", tag="gsum")
                    nc.gpsimd.tensor_add(t["gsum"][:], t["m1"][:], t["m2"][:])

                def gs8(t, tok0):
                    t["gs8"] = qpool.tile([128, 8], f32, name="gs8", tag="gs8")
                    nc.vector.max(t["gs8"][:], t["gsum"][:])

                def pen(t, tok0):
                    # pen = (gsum < 4th) * -1e30: -0.0 for kept groups, so kept
                    # scores pass through bit-exact
                    t["pen"] = qpool.tile([128, G], f32, name="pen", tag="pen")
                    nc.vector.tensor_scalar(t["pen"][:], t["gsum"][:],
                                            t["gs8"][:, 3:4], -1e30,
                                            op0=OP.is_lt, op1=OP.mult)

                def s_mask(t, tok0):
                    t["s_mask"] = mpool.tile([128, E], f32, name="s_mask", tag="s_mask")
                    pen_b = t["pen"][:].unsqueeze(2).broadcast_to([128, G, GS])
                    nc.gpsimd.tensor_tensor(
                        t["s_mask"][:].rearrange("p (g s) -> p g s", g=G),
                        t["s_cho"][:].rearrange("p (g s) -> p g s", g=G),
                        pen_b, op=OP.add)

                def v8(t, tok0):
                    t["v8"] = qpool.tile([128, 8], f32, name="v8", tag="v8")
                    nc.vector.max(t["v8"][:], t["s_mask"][:])

                def i8(t, tok0):
                    t["i8"] = qpool.tile([128, 8], u32, name="i8", tag="i8")
                    nc.vector.max_index(t["i8"][:], t["v8"][:], t["s_mask"][:])
                    nc.gpsimd.dma_start(idx_out[tok0:tok0 + 128, :], t["i8"][:])

                def sel2(t, tok0):
                    t["sel2"] = mpool.tile([128, E], f32, name="sel2", tag="sel2")
                    nc.vector.tensor_scalar(t["sel2"][:], t["s_mask"][:],
                                            t["v8"][:, 7:8], -1.0,
                                            op0=OP.is_ge, op1=OP.add)

                def r_sel(t, tok0):
                    # raw scores (psT) masked to the selected 8
                    t["r_sel"] = mpool.tile([128, E], f32, name="r_sel", tag="r_sel")
                    nc.vector.scalar_tensor_tensor(t["r_sel"][:], in0=t["sel2"][:],
                                                   scalar=1e30, in1=t["psT"][:],
                                                   op0=OP.mult, op1=OP.add)

                def w8d(t, tok0):
                    t["w8d"] = qpool.tile([128, 8], f32, name="w8d", tag="w8d")
                    nc.vector.max(t["w8d"][:], t["r_sel"][:])

                def ri8(t, tok0):
                    t["ri8"] = qpool.tile([128, 8], u32, name="ri8", tag="ri8")
                    nc.vector.max_index(t["ri8"][:], t["w8d"][:], t["r_sel"][:])

                def eq64(t, tok0):
                    t["eq64"] = qpool.tile([128, 8, 8], f32, name="eq64", tag="eq64")
                    i8_b = t["i8"][:].unsqueeze(2).broadcast_to([128, 8, 8])
                    ri8_b = t["ri8"][:].unsqueeze(1).broadcast_to([128, 8, 8])
                    nc.vector.tensor_tensor(t["eq64"][:], i8_b, ri8_b,
                                            op=OP.is_equal)

                def w64(t, tok0):
                    t["w64"] = qpool.tile([128, 8, 8], f32, name="w64", tag="w64")
                    w8d_b = t["w8d"][:].unsqueeze(1).broadcast_to([128, 8, 8])
                    nc.gpsimd.tensor_tensor(t["w64"][:], t["eq64"][:], w8d_b,
                                            op=OP.mult)

                def w8p(t, tok0):
                    t["w8p"] = qpool.tile([128, 8], f32, name="w8p", tag="w8p")
                    nc.vector.reduce_sum(t["w8p"][:], t["w64"][:], axis=AX.X)

                def sum8(t, tok0):
                    t["sum8"] = qpool.tile([128, 1], f32, name="sum8", tag="sum8")
                    nc.vector.reduce_sum(t["sum8"][:], t["w8p"][:], axis=AX.X)

                def rcp(t, tok0):
                    t["rcp"] = qpool.tile([128, 1], f32, name="rcp", tag="rcp")
                    nc.vector.reciprocal(t["rcp"][:], t["sum8"][:])

                def wf(t, tok0):
                    t["wf"] = qpool.tile([128, 8], f32, name="wf", tag="wf")
                    nc.vector.tensor_scalar(t["wf"][:], t["w8p"][:],
                                            t["rcp"][:, 0:1], SCALING,
                                            op0=OP.mult, op1=OP.mult)
                    nc.gpsimd.dma_start(w_out[tok0:tok0 + 128, :], t["wf"][:])

                for fn in (s_cho, m1, repl, m2, gsum, gs8, pen, s_mask, v8, i8,
                           sel2, r_sel, w8d, ri8, eq64, w64, w8p, sum8, rcp, wf):
                    step(fn)

            # software pipeline: routing for block b-1 is emitted BEFORE block
            # b's matmuls, so the b-1 transposes sit ahead of MM(b) in the
            # in-order PE queue and routing overlaps the MM stream
            for bi in range(NB + 1):
                if bi >= 1:
                    emit_routing_block(bi - 1)
                if bi < NB:
                    emit_mm_block(bi)

    nc.compile()
    return nc


def _get_built():
    global _BUILT
    if _BUILT is None:
        _BUILT = _build()
    return _BUILT


def _e4m3(a):
    return np.clip(a, -240.0, 240.0).astype(F8)


def _prep_in_maps(hidden_states, weight, e_score_correction_bias):
    x = np.asarray(hidden_states, dtype=np.float32).reshape(T_FULL, H)
    W = np.asarray(weight, dtype=np.float32)
    b = np.asarray(e_score_correction_bias, dtype=np.float32)

    # ---- weights (shared by all cores) ----
    Wt = np.ascontiguousarray(W.T)                       # [H, E]
    w1 = Wt.astype(np.float16)
    w1f = w1.astype(np.float32)
    w2 = Wt - w1f
    w1s_full = (w1f * 256.0).astype(np.float16)          # exact pow2 scale
    # [H, E] -> [128, c, eh, e]
    w1s_host = np.ascontiguousarray(
        w1s_full.reshape(NCHK, 128, EH, 128).transpose(1, 0, 2, 3)
    ).reshape(128, NCHK * EH * 128)
    wc8_pl0 = _e4m3(w2 * np.float32(2.0 ** 16))          # [H, E]
    wc8_pl1 = _e4m3(w1f * np.float32(2.0 ** 5))
    wc8_st = np.stack([wc8_pl0.reshape(NCHK, 128, EH, 128),
                       wc8_pl1.reshape(NCHK, 128, EH, 128)], axis=3)  # c,p,eh,l,e
    wc8_host = np.ascontiguousarray(wc8_st.transpose(1, 0, 2, 3, 4)).reshape(
        128, NCHK * EH * 2 * 128)

    bias_rep = np.ascontiguousarray(np.tile(b[None, :], (128, 1)))
    ident = np.eye(128, dtype=np.float32)

    # ---- per-core x ----
    in_maps = []
    for core in range(N_CORES):
        sl = slice(core * T_CORE, (core + 1) * T_CORE)
        xT = np.ascontiguousarray(x[sl].T)               # [H, 2048]
        x1 = xT.astype(np.float16)
        x1f = x1.astype(np.float32)
        x2 = xT - x1f
        x1s_full = (x1f * 256.0).astype(np.float16)      # [H, 2048]
        x1_8 = _e4m3(x1f)
        x2_8 = _e4m3(x2 * np.float32(2.0 ** 11))

        # [H, T] -> [NB, 128, c*t] (block-major, chunk-major within block)
        v = x1s_full.reshape(NCHK, 128, NB, TB)
        x1s_host = np.ascontiguousarray(v.transpose(2, 1, 0, 3)).reshape(
            NB, 128, NCHK * TB)
        v8 = np.stack([x1_8.reshape(NCHK, 128, NB, TB),
                       x2_8.reshape(NCHK, 128, NB, TB)], axis=1)  # c,l,p,b,t
        xc8_host = np.ascontiguousarray(v8.transpose(3, 2, 0, 1, 4)).reshape(
            NB, 128, NCHK * 2 * TB)

        in_maps.append({
            "x1s": x1s_host, "xc8": xc8_host,
            "w1s": w1s_host, "wc8": wc8_host,
            "bias_rep": bias_rep, "ident": ident,
        })
    return in_maps


def kernel(hidden_states: np.ndarray, weight: np.ndarray,
           e_score_correction_bias: np.ndarray):
    in_maps = _prep_in_maps(hidden_states, weight, e_score_correction_bias)
    nc = _get_built()
    res = run_bass_kernel_spmd(nc, in_maps, list(range(N_CORES)))

    idx = np.concatenate([r["idx_out"] for r in res.results], axis=0).astype(np.int32)
    w = np.concatenate([r["w_out"] for r in res.results], axis=0).astype(np.float32)
    return idx, w
